# revision 25
# baseline (speedup 1.0000x reference)
"""Trainium2 Bass kernel for a 2-layer GCN (Cora-style GNN message passing).

Computation (see reference):
    S1 = x @ W1                      # [N, 40]
    agg1[d] = sum_e w_e * S1[src_e]  (segment-sum over dst) + b1
    h = relu(agg1) * keep            # keep = (dropout_mask > 0.5) / 0.5
    out = log_softmax((A @ h) @ W2 + b2)   # reassociated: agg2 = A@h, then @W2

Distribution (8 NeuronCores): nodes are sharded by dst range; each core owns
12,500 nodes (padded to 12,800) and all edges whose dst falls in its range.
Each core computes S1 rows for its own nodes, the [102400, 40] bf16 tables
are all-gathered, and each per-core segment-sum is an indirect-DMA gather of
src rows plus one-hot matmuls on the tensor engine:

  - edges are sorted by dst and packed into groups of 128 (partition dim),
    each group confined to a 32-dst window,
  - the "weighted one-hot" [128 edges, 32 slots] bf16 is built ON DEVICE from
    per-edge (slot, weight) arrays via an is_equal + multiply on the DVE,
  - layer 1 accumulates node-major [32, 4, 40] PSUM tiles; layer 2 flips the
    matmul operands to produce hid-major [40, 4, 32] tiles that feed the
    final @W2 matmul directly (no tensor-engine transposes anywhere).

x is sent in natural [node, feat] layout as bf16 (cheap host bit-trick cast)
and transposed on device by the DMA xbar. All inputs are kept as small as
possible: the dominant cost in this environment is host->device transfer of
the inputs, not device execution. All group counts are unified across cores
so the single SPMD program works on every core; padding edges carry
weight 0.
"""

import os
import numpy as np
from dataclasses import dataclass


@dataclass(frozen=True)
class Cfg:
    ncores: int = 8
    own: int = 12500          # real nodes per core
    nodes: int = 12800        # padded nodes per core (multiple of 128)
    feat: int = 1433
    fpad: int = 1408          # feat rounded down to multiple of 128
    ftail: int = 25           # remaining features (sent pre-transposed)
    hid: int = 40
    ncls: int = 7
    win: int = 32             # dst nodes per window (one-hot width)
    wpt: int = 4              # windows per 128-node tile (128/win)
    grp: int = 512            # phase-A node group (per DMA-transpose batch)

    @property
    def tiles(self):
        return self.nodes // 128

    @property
    def windows(self):
        return self.nodes // self.win  # per core

    @property
    def kt(self):
        return self.fpad // 128

    @property
    def n(self):
        return self.ncores * self.own

    @property
    def table_rows(self):
        return self.ncores * self.nodes


CFG = Cfg()


# --------------------------------------------------------------------------
# Host-side preprocessing
# --------------------------------------------------------------------------

def _bf16_trunc_bits(a_f32):
    """bf16 bit pattern of a float32 array via truncation (no arithmetic).

    This numpy build has pathologically slow dtype-cast loops (~30 MB/s) but
    fast same-dtype strided copies, so all bf16 conversion is done with
    uint16 byte views. Truncation costs <=1 ulp (0.4% rel) vs round-to-
    nearest -- well within the error budget.
    """
    a = np.ascontiguousarray(a_f32, dtype=np.float32)
    return a.view(np.uint16).reshape(*a.shape[:-1], a.shape[-1] * 2)[
        ..., 1::2  # little-endian: high half-word of each f32
    ]


def host_prep(cfg, x, src, dst, edge_weight, W1, b1, W2, b2, dropout_mask_u):
    """Build per-core input arrays + the (core-invariant) group structure."""
    import ml_dtypes

    bf16 = ml_dtypes.bfloat16
    ncores, own, nodes, win, wpt = cfg.ncores, cfg.own, cfg.nodes, cfg.win, cfg.wpt
    windows, hid, tiles = cfg.windows, cfg.hid, cfg.tiles

    # ---- edge structure (sorted by dst window, packed into 128-edge groups)
    dst = np.ascontiguousarray(dst, dtype=np.int32)
    src = np.ascontiguousarray(src, dtype=np.int32)
    core = dst // own
    ldst = dst - core * own
    wloc = ldst // win
    slot = ldst - wloc * win                 # [0, win)
    gwin = core * windows + wloc             # global window id

    nwin_total = ncores * windows
    cnt_flat = np.bincount(gwin, minlength=nwin_total)
    cnt = cnt_flat.reshape(ncores, windows)
    Gw = np.maximum(1, -(-cnt // 128)).max(axis=0)          # [windows]
    woff = np.concatenate([[0], np.cumsum(Gw)]).astype(np.int64)
    G = int(woff[-1])

    order = np.argsort(gwin, kind="stable")
    gw_sorted = gwin[order]
    grp_start = np.concatenate([[0], np.cumsum(cnt_flat)])
    pos_in_win = np.arange(len(src), dtype=np.int64) - grp_start[gw_sorted]
    tgt = woff[gw_sorted % windows] * 128 + pos_in_win       # per-core slot
    c_sorted = gw_sorted // windows

    # table row of a src node (tables are per-core blocks of `nodes` rows)
    sc = src // own
    src_row = sc * nodes + (src - sc * own)

    # pack (slot, weight) into one uint16: slot<<11 | floor(w*2048).
    # Device dequant: w ~ (wq + 0.5)/2048, abs err <= 1/4096.
    flat = (c_sorted * (G * 128) + tgt).astype(np.int64)
    idx_all = np.zeros(ncores * G * 128, np.int32)
    idx_all[flat] = src_row[order]
    wq = (edge_weight * np.float32(2048.0)).astype(np.int32)
    sw = (slot << 11) | wq
    swp_all = np.zeros(ncores * G * 128, np.uint16)
    swp_all[flat] = sw[order]

    gidx = np.ascontiguousarray(
        idx_all.reshape(ncores, G, 128).transpose(0, 2, 1))
    swp = np.ascontiguousarray(
        swp_all.reshape(ncores, G, 128).transpose(0, 2, 1))

    # ---- x: natural layout, bf16 (truncation); the 25-feature tail is sent
    # pre-transposed so device feat-chunks stay multiples of 128
    xbits = _bf16_trunc_bits(x).reshape(ncores, own, cfg.feat)
    xn = np.zeros((ncores, nodes, cfg.fpad), np.uint16)
    xn[:, :own, :] = xbits[:, :, : cfg.fpad]
    xn = xn.view(bf16)
    xtailT = np.zeros((ncores, cfg.ftail, nodes), np.uint16)
    xtailT[:, :, :own] = xbits[:, :, cfg.fpad:].transpose(0, 2, 1)
    xtailT = xtailT.view(bf16)

    # ---- keep mask (0 or 2) as uint8, [tiles, 32 slot, 4 win, 40] layout
    kp = np.zeros((ncores, nodes, hid), np.uint8)
    kb = (dropout_mask_u > 0.5).view(np.uint8)
    kp[:, :own] = (kb + kb).reshape(ncores, own, hid)
    keep4 = np.ascontiguousarray(
        kp.reshape(ncores, tiles, wpt, win, hid).transpose(0, 1, 3, 2, 4)
    ).reshape(ncores, tiles, win, wpt * hid)

    # ---- weights / consts (small; any cast path is fine)
    w1p = np.ascontiguousarray(
        W1[: cfg.fpad].reshape(cfg.kt, 128, hid).transpose(1, 0, 2)
    ).astype(bf16)
    w1t = np.ascontiguousarray(W1[cfg.fpad:]).astype(bf16)
    w2 = W2.astype(np.float32)
    b1r = np.broadcast_to(
        b1.astype(np.float32), (win, 1, hid)).copy()
    b2r = np.broadcast_to(
        b2.astype(np.float32), (128, 1, cfg.ncls)).copy()
    vslot = np.broadcast_to(
        np.arange(win, dtype=np.float32).astype(bf16), (128, 1, win)).copy()

    in_maps = [
        {
            "xn": xn[k],
            "xtailT": xtailT[k],
            "w1p": w1p,
            "w1t": w1t,
            "w2": w2,
            "b1r": b1r,
            "b2r": b2r,
            "vslot": vslot,
            "keep4": keep4[k],
            "gidx": gidx[k],
            "swp": swp[k],
        }
        for k in range(ncores)
    ]
    return in_maps, Gw


# --------------------------------------------------------------------------
# Numpy emulation of the device algorithm (for validation)
# --------------------------------------------------------------------------

def emulate(cfg, in_maps, Gw):
    import ml_dtypes
    f32, f16 = np.float32, ml_dtypes.bfloat16
    ncores, nodes, win, wpt = cfg.ncores, cfg.nodes, cfg.win, cfg.wpt
    hid, ncls, tiles = cfg.hid, cfg.ncls, cfg.tiles
    G = int(Gw.sum())
    woff = np.concatenate([[0], np.cumsum(Gw)])

    # phase A: S1 tables (natural row order)
    s1 = np.zeros((ncores, nodes, hid), f16)
    for k in range(ncores):
        xk = in_maps[k]["xn"].astype(f32)     # [nodes, fpad]
        w1p = in_maps[k]["w1p"].astype(f32)   # [128, kt, hid]
        w1 = w1p.transpose(1, 0, 2).reshape(cfg.fpad, hid)
        xt = in_maps[k]["xtailT"].astype(f32)  # [ftail, nodes]
        w1t = in_maps[k]["w1t"].astype(f32)    # [ftail, hid]
        s1[k] = (xk @ w1 + xt.T @ w1t).astype(f16)
    s1_full = s1.reshape(ncores * nodes, hid)

    def build_onehot(k):
        swp = in_maps[k]["swp"].astype(np.int64)  # [128, G]
        slotb = (swp >> 11).astype(f32)
        wgt = (((swp & 0x7FF).astype(f32) + 0.5) / 2048.0).astype(f16)
        wgt = wgt.astype(f32)
        oh = (slotb[:, :, None] == np.arange(win)[None, None, :])
        return (oh * wgt[:, :, None]).astype(f16).astype(f32)  # [128, G, win]

    def spmm(table, k, oh):
        gidx = in_maps[k]["gidx"]                 # [128, G]
        msg = table[gidx.T].astype(f32)           # [G, 128, hid]
        ohg = oh.transpose(1, 0, 2)               # [G, 128, win]
        agg = np.zeros((tiles, win, wpt, hid), f32)
        for w in range(cfg.windows):
            t, wl = divmod(w, wpt)
            for g in range(woff[w], woff[w + 1]):
                agg[t, :, wl, :] += ohg[g].T @ msg[g]
        return agg                                # [tiles, 32s, 4w, hid]

    h = np.zeros((ncores, nodes, hid), f16)
    for k in range(ncores):
        oh = build_onehot(k)
        agg1 = spmm(s1_full, k, oh)
        b1 = in_maps[k]["b1r"][0, 0]
        keep = in_maps[k]["keep4"].reshape(tiles, win, wpt, hid)
        hb = np.maximum(agg1 + b1, 0.0).astype(f16).astype(f32) * keep
        # natural row order: node (t, w, s) lives at hb[t, s, w]
        h[k] = hb.transpose(0, 2, 1, 3).reshape(nodes, hid).astype(f16)
        in_maps[k]["_oh"] = oh
    h_full = h.reshape(ncores * nodes, hid)

    outs = []
    for k in range(ncores):
        agg2 = spmm(h_full, k, in_maps[k]["_oh"])   # [tiles, 32s, 4w, hid]
        # natural node order: node (t, w, s) -> agg2[t, s, w]
        aggn = agg2.transpose(0, 2, 1, 3).reshape(nodes, hid)
        z = aggn @ in_maps[k]["w2"] + in_maps[k]["b2r"][0, 0]
        m = z.max(1, keepdims=True)
        out = (z - m) - np.log(np.exp(z - m).sum(1, keepdims=True))
        outs.append(out[: cfg.own])
        del in_maps[k]["_oh"]
    return np.concatenate(outs).astype(np.float32)


# --------------------------------------------------------------------------
# Bass/Tile program
# --------------------------------------------------------------------------

def build_program(cfg, Gw, num_devices):
    import concourse.bass as bass
    import concourse.bacc as bacc
    import concourse.mybir as mybir
    import concourse.tile as tile

    f32 = mybir.dt.float32
    bf = mybir.dt.bfloat16
    i32 = mybir.dt.int32
    u16 = mybir.dt.uint16
    u8 = mybir.dt.uint8
    AF = mybir.ActivationFunctionType
    OP = mybir.AluOpType
    X = mybir.AxisListType.X

    G = int(Gw.sum())
    woff = np.concatenate([[0], np.cumsum(Gw)])
    nodes, tiles, win, wpt = cfg.nodes, cfg.tiles, cfg.win, cfg.wpt
    hid, ncls, kt, grp = cfg.hid, cfg.ncls, cfg.kt, cfg.grp
    trows = num_devices * nodes

    nc = bacc.Bacc(
        "TRN2", target_bir_lowering=False, debug=False,
        num_devices=num_devices,
    )

    xn = nc.dram_tensor("xn", [nodes, cfg.fpad], bf, kind="ExternalInput")
    xtailT = nc.dram_tensor(
        "xtailT", [cfg.ftail, nodes], bf, kind="ExternalInput")
    w1p = nc.dram_tensor("w1p", [128, kt, hid], bf, kind="ExternalInput")
    w1t = nc.dram_tensor("w1t", [cfg.ftail, hid], bf, kind="ExternalInput")
    w2 = nc.dram_tensor("w2", [hid, ncls], f32, kind="ExternalInput")
    b1r = nc.dram_tensor("b1r", [win, 1, hid], f32, kind="ExternalInput")
    b2r = nc.dram_tensor("b2r", [128, 1, ncls], f32, kind="ExternalInput")
    vslot = nc.dram_tensor("vslot", [128, 1, win], bf, kind="ExternalInput")
    keep4 = nc.dram_tensor(
        "keep4", [tiles, win, wpt * hid], u8, kind="ExternalInput")
    gidx = nc.dram_tensor("gidx", [128, G], i32, kind="ExternalInput")
    swp = nc.dram_tensor("swp", [128, G], u16, kind="ExternalInput")
    out_d = nc.dram_tensor("out", [nodes, ncls], f32, kind="ExternalOutput")

    s1_own = nc.dram_tensor("s1_own", [nodes, hid], bf)
    s1_full = nc.dram_tensor("s1_full", [trows, hid], bf, addr_space="Shared")
    h_own = nc.dram_tensor("h_own", [nodes, hid], bf)
    h_full = nc.dram_tensor("h_full", [trows, hid], bf, addr_space="Shared")

    groups = list(range(num_devices))

    # per-tile group schedule: (g_global, window_in_tile, start, stop)
    sched = []
    for t in range(tiles):
        entries = []
        for wl in range(wpt):
            w = t * wpt + wl
            for g in range(woff[w], woff[w + 1]):
                entries.append(
                    (int(g), wl, g == woff[w], g == woff[w + 1] - 1)
                )
        sched.append(entries)
    rmax = int(max(woff[(t + 1) * wpt] - woff[t * wpt] for t in range(tiles)))

    ngrp = nodes // grp
    spg = grp // 128  # 128-node subtiles per phase-A group

    with tile.TileContext(nc) as tc:
        with (
            tc.tile_pool(name="const", bufs=1) as constp,
            tc.tile_pool(name="xbuf", bufs=3) as xpool,
            tc.tile_pool(name="psA", bufs=2, space="PSUM") as psA,
            tc.tile_pool(name="s1pc", bufs=3) as spool,
            tc.tile_pool(name="msg", bufs=3) as msgp,
            tc.tile_pool(name="oh", bufs=3) as ohp,
            tc.tile_pool(name="psB", bufs=2, space="PSUM") as psB,
            tc.tile_pool(name="hb", bufs=3) as hpool,
            tc.tile_pool(name="psC", bufs=2, space="PSUM") as psC,
            tc.tile_pool(name="ps2", bufs=2, space="PSUM") as ps2,
            tc.tile_pool(name="ob", bufs=3) as opool,
        ):
            # ---- constants + resident metadata ----
            w1sb = constp.tile([128, kt, hid], bf)
            nc.sync.dma_start(out=w1sb[:], in_=w1p[:])
            w1tsb = constp.tile([cfg.ftail, hid], bf)
            nc.sync.dma_start(out=w1tsb[:], in_=w1t[:])
            xtl = constp.tile([cfg.ftail, nodes], bf)
            nc.sync.dma_start(out=xtl[:], in_=xtailT[:])
            w2sb = constp.tile([hid, ncls], f32)
            nc.sync.dma_start(out=w2sb[:], in_=w2[:])
            b1sb = constp.tile([win, 1, hid], f32)
            nc.sync.dma_start(out=b1sb[:], in_=b1r[:])
            b2sb = constp.tile([128, 1, ncls], f32)
            nc.sync.dma_start(out=b2sb[:], in_=b2r[:])
            vs = constp.tile([128, 1, win], bf)
            nc.sync.dma_start(out=vs[:], in_=vslot[:])
            gix = constp.tile([128, G], i32)
            nc.sync.dma_start(out=gix[:], in_=gidx[:])
            # unpack swp = slot<<11 | floor(w*2048) into resident bf16 arrays
            swp_sb = constp.tile([128, G], u16)
            nc.sync.dma_start(out=swp_sb[:], in_=swp[:])
            tmp16 = constp.tile([128, G], u16)
            slb = constp.tile([128, G], bf)
            wgb = constp.tile([128, G], bf)
            nc.vector.tensor_scalar(
                out=tmp16[:], in0=swp_sb[:], scalar1=11, scalar2=None,
                op0=OP.logical_shift_right)
            nc.vector.tensor_copy(slb[:], tmp16[:])
            nc.vector.tensor_scalar(
                out=tmp16[:], in0=swp_sb[:], scalar1=0x7FF, scalar2=None,
                op0=OP.bitwise_and)
            nc.scalar.activation(
                out=wgb[:], in_=tmp16[:], func=AF.Copy,
                scale=1.0 / 2048.0, bias=0.5 / 2048.0)
            # keep mask: u8 -> resident bf16 [32, tiles, 4, 40]
            kp8 = constp.tile([win, tiles, wpt * hid], u8)
            nc.sync.dma_start(
                out=kp8[:], in_=keep4[:].rearrange("t p c -> p t c"))
            kpb = constp.tile([win, tiles, wpt, hid], bf)
            nc.vector.tensor_copy(
                kpb[:], kp8[:].rearrange("p t (w c) -> p t w c", w=wpt))

            # ---- phase A: S1_own = (x @ W1), row-permuted store ----
            for c in range(ngrp):
                xt = xpool.tile([128, kt, grp], bf)
                for k in range(kt):
                    nc.sync.dma_start(
                        out=xt[:, k, :],
                        in_=xn[c * grp:(c + 1) * grp, k * 128:(k + 1) * 128],
                        transpose=True,
                    )
                for sub in range(spg):
                    t_n0 = c * grp + sub * 128
                    ps = psA.tile([128, hid], f32)
                    for k in range(kt):
                        nc.tensor.matmul(
                            ps[:],
                            lhsT=xt[:, k, sub * 128:(sub + 1) * 128],
                            rhs=w1sb[:, k, :],
                            start=(k == 0), stop=False,
                        )
                    nc.tensor.matmul(
                        ps[:],
                        lhsT=xtl[:, t_n0:t_n0 + 128],
                        rhs=w1tsb[:],
                        start=False, stop=True,
                    )
                    pc = spool.tile([128, hid], bf, tag="s1pc")
                    nc.vector.tensor_copy(pc[:], ps[:])
                    t_ = c * spg + sub
                    nc.sync.dma_start(
                        out=s1_own[t_ * 128:(t_ + 1) * 128, :], in_=pc[:]
                    )

            # ---- all-gather S1 ----
            nc.gpsimd.collective_compute(
                "AllGather", OP.bypass, replica_groups=[groups],
                ins=[s1_own[:]], outs=[s1_full[:]],
            )

            def gather_and_onehot(t, table, mtag, otag):
                r0 = int(woff[t * wpt])
                rt = int(woff[(t + 1) * wpt]) - r0
                msg = msgp.tile([128, rmax, hid], bf, tag=mtag)
                # funnel the gather's dependencies (WAR on msg) through
                # cheap Pool-engine ops first
                scr = spool.tile([1, 1], i32, tag="scr")
                nc.gpsimd.tensor_copy(scr[:], gix[:1, :1])
                nc.gpsimd.memset(msg[:1, :1, :1], 0.0)
                # HW only supports one offset per partition per indirect DMA
                for r in range(rt):
                    nc.gpsimd.indirect_dma_start(
                        out=msg[:, r, :], out_offset=None,
                        in_=table[:],
                        in_offset=bass.IndirectOffsetOnAxis(
                            ap=gix[:, r0 + r:r0 + r + 1], axis=0
                        ),
                    )
                oht = ohp.tile([128, rmax, win], bf, tag=otag)
                nc.vector.tensor_tensor(
                    out=oht[:, :rt, :],
                    in0=slb[:, r0:r0 + rt].to_broadcast([128, rt, win]),
                    in1=vs[:].to_broadcast([128, rt, win]),
                    op=OP.is_equal,
                )
                nc.vector.tensor_tensor(
                    out=oht[:, :rt, :],
                    in0=oht[:, :rt, :],
                    in1=wgb[:, r0:r0 + rt].to_broadcast([128, rt, win]),
                    op=OP.mult,
                )
                return r0, msg, oht

            # ---- layer 1 SpMM -> h (node-major psum) ----
            for t in range(tiles):
                r0, msg, oht = gather_and_onehot(t, s1_full, "msg1", "oh1")
                ps = psB.tile([win, wpt, hid], f32, tag="agg")
                for (g, wl, st, sp) in sched[t]:
                    r = g - r0
                    nc.tensor.matmul(
                        ps[:, wl, :],
                        lhsT=oht[:, r, :], rhs=msg[:, r, :],
                        start=st, stop=sp,
                    )
                hb = hpool.tile([win, wpt, hid], f32, tag="hb")
                nc.vector.tensor_tensor(
                    out=hb[:], in0=ps[:],
                    in1=b1sb[:].to_broadcast([win, wpt, hid]), op=OP.add,
                )
                nc.scalar.activation(out=hb[:], in_=hb[:], func=AF.Relu)
                hf = hpool.tile([win, wpt, hid], bf, tag="hf")
                nc.vector.tensor_tensor(
                    out=hf[:], in0=hb[:], in1=kpb[:, t], op=OP.mult,
                )
                nc.sync.dma_start(
                    out=h_own[t * 128:(t + 1) * 128, :]
                    .rearrange("(w s) c -> s w c", w=wpt, s=win),
                    in_=hf[:],
                )

            # ---- all-gather h ----
            nc.gpsimd.collective_compute(
                "AllGather", OP.bypass, replica_groups=[groups],
                ins=[h_own[:]], outs=[h_full[:]],
            )

            # ---- layer 2 SpMM (hid-major psum) + @W2 + log_softmax ----
            for t in range(tiles):
                r0, msg, oht = gather_and_onehot(t, h_full, "msg2", "oh2")
                pst = psC.tile([hid, wpt, win], f32, tag="aggT")
                for (g, wl, st, sp) in sched[t]:
                    r = g - r0
                    nc.tensor.matmul(
                        pst[:, wl, :],
                        lhsT=msg[:, r, :], rhs=oht[:, r, :],
                        start=st, stop=sp,
                    )
                at = hpool.tile([hid, wpt, win], f32, tag="at")
                nc.vector.tensor_copy(at[:], pst[:])
                p2 = ps2.tile([128, 1, ncls], f32, tag="s2")
                nc.tensor.matmul(
                    p2[:, 0, :], lhsT=at[:].rearrange("p w s -> p (w s)"),
                    rhs=w2sb[:], start=True, stop=True,
                )
                z = opool.tile([128, 1, ncls], f32, tag="z")
                nc.vector.tensor_tensor(
                    out=z[:], in0=p2[:], in1=b2sb[:], op=OP.add,
                )
                m = opool.tile([128, 1], f32, tag="m")
                nc.vector.tensor_reduce(out=m[:], in_=z[:], axis=X, op=OP.max)
                zc = opool.tile([128, 1, ncls], f32, tag="zc")
                nc.vector.tensor_tensor(
                    out=zc[:], in0=z[:],
                    in1=m[:].to_broadcast([128, 1, ncls]), op=OP.subtract,
                )
                ez = opool.tile([128, 1, ncls], f32, tag="ez")
                nc.scalar.activation(out=ez[:], in_=zc[:], func=AF.Exp)
                s = opool.tile([128, 1], f32, tag="s")
                nc.vector.tensor_reduce(out=s[:], in_=ez[:], axis=X, op=OP.add)
                ls = opool.tile([128, 1], f32, tag="ls")
                nc.scalar.activation(out=ls[:], in_=s[:], func=AF.Ln)
                res = opool.tile([128, 1, ncls], f32, tag="res")
                nc.vector.tensor_tensor(
                    out=res[:], in0=zc[:],
                    in1=ls[:].to_broadcast([128, 1, ncls]), op=OP.subtract,
                )
                nc.sync.dma_start(
                    out=out_d[t * 128:(t + 1) * 128, :], in_=res[:, 0, :]
                )

    nc.compile()
    return nc


# --------------------------------------------------------------------------
# Entry point
# --------------------------------------------------------------------------

def kernel(x, src, dst, edge_weight, W1, b1, W2, b2, dropout_mask_u):
    cfg = CFG
    in_maps, Gw = host_prep(
        cfg, x, src, dst, edge_weight, W1, b1, W2, b2, dropout_mask_u
    )
    nc = build_program(cfg, Gw, cfg.ncores)

    from concourse.bass_utils import run_bass_kernel_spmd

    trace = bool(int(os.environ.get("GNN_TRACE", "0")))
    try:
        res = run_bass_kernel_spmd(
            nc, in_maps, core_ids=list(range(cfg.ncores)), trace=trace
        )
    except ModuleNotFoundError:
        res = run_bass_kernel_spmd(
            nc, in_maps, core_ids=list(range(cfg.ncores)), trace=False
        )
    kernel.last_exec_time_ns = getattr(res, "exec_time_ns", None)
    kernel.last_profile = res
    kernel.last_nc = nc
    kernel.last_in_maps = in_maps
    out = np.concatenate(
        [res.results[k]["out"][: cfg.own] for k in range(cfg.ncores)]
    )
    return out.astype(np.float32)


# revision 32
# speedup vs baseline: 1.9969x; 1.9969x over previous
"""Trainium2 Bass kernel for a 2-layer GCN (Cora-style GNN message passing).

Computation (see reference):
    S1 = x @ W1                      # [N, 40]
    agg1[d] = sum_e w_e * S1[src_e]  (segment-sum over dst) + b1
    h = relu(agg1) * keep            # keep = (dropout_mask > 0.5) / 0.5
    out = log_softmax((A @ h) @ W2 + b2)   # reassociated: agg2 = A@h, then @W2

Distribution (8 NeuronCores): nodes are sharded by dst range; each core owns
12,500 nodes (padded to 12,800) and all edges whose dst falls in its range.
Each core computes S1 rows for its own nodes, the [102400, 40] bf16 tables
are all-gathered, and each per-core segment-sum is an indirect-DMA gather of
src rows plus one-hot matmuls on the tensor engine:

  - edges are sorted by dst and packed into groups of 128 (partition dim),
    each group confined to a 32-dst window,
  - the "weighted one-hot" [128 edges, 32 slots] bf16 is built ON DEVICE from
    per-edge (slot, weight) arrays via an is_equal + multiply on the DVE,
  - layer 1 accumulates node-major [32, 4, 40] PSUM tiles; layer 2 flips the
    matmul operands to produce hid-major [40, 4, 32] tiles that feed the
    final @W2 matmul directly (no tensor-engine transposes anywhere).

x is sent in natural [node, feat] layout as bf16 (cheap host bit-trick cast)
and transposed on device by the DMA xbar. All inputs are kept as small as
possible: the dominant cost in this environment is host->device transfer of
the inputs, not device execution. All group counts are unified across cores
so the single SPMD program works on every core; padding edges carry
weight 0.
"""

import os
import numpy as np
from dataclasses import dataclass


@dataclass(frozen=True)
class Cfg:
    ncores: int = 8
    own: int = 12500          # real nodes per core
    nodes: int = 12800        # padded nodes per core (multiple of 128)
    feat: int = 1433
    fpad: int = 1408          # feat rounded down to multiple of 128
    ftail: int = 25           # remaining features (sent pre-transposed)
    nsend: int = 12512        # x rows sent per core (own rounded up to 16)
    hid: int = 40
    ncls: int = 7
    win: int = 32             # dst nodes per window (one-hot width)
    wpt: int = 4              # windows per 128-node tile (128/win)
    grp: int = 512            # phase-A node group (per DMA-transpose batch)

    @property
    def tiles(self):
        return self.nodes // 128

    @property
    def windows(self):
        return self.nodes // self.win  # per core

    @property
    def kt(self):
        return self.fpad // 128

    @property
    def n(self):
        return self.ncores * self.own

    @property
    def table_rows(self):
        return self.ncores * self.nodes


CFG = Cfg()


# --------------------------------------------------------------------------
# Host-side preprocessing
# --------------------------------------------------------------------------

def _bf16_trunc_bits(a_f32):
    """bf16 bit pattern of a float32 array via truncation (no arithmetic).

    This numpy build has pathologically slow dtype-cast loops (~30 MB/s) but
    fast same-dtype strided copies, so all bf16 conversion is done with
    uint16 byte views. Truncation costs <=1 ulp (0.4% rel) vs round-to-
    nearest -- well within the error budget.
    """
    a = np.ascontiguousarray(a_f32, dtype=np.float32)
    return a.view(np.uint16).reshape(*a.shape[:-1], a.shape[-1] * 2)[
        ..., 1::2  # little-endian: high half-word of each f32
    ]


def host_prep(cfg, x, src, dst, edge_weight, W1, b1, W2, b2, dropout_mask_u):
    """Build per-core input arrays + the (core-invariant) group structure."""
    import ml_dtypes

    bf16 = ml_dtypes.bfloat16
    ncores, own, nodes, win, wpt = cfg.ncores, cfg.own, cfg.nodes, cfg.win, cfg.wpt
    windows, hid, tiles = cfg.windows, cfg.hid, cfg.tiles

    # ---- edge structure (sorted by dst window, packed into 128-edge groups)
    dst = np.ascontiguousarray(dst, dtype=np.int32)
    src = np.ascontiguousarray(src, dtype=np.int32)
    core = dst // own
    ldst = dst - core * own
    wloc = ldst // win
    slot = ldst - wloc * win                 # [0, win)
    gwin = core * windows + wloc             # global window id

    nwin_total = ncores * windows
    cnt_flat = np.bincount(gwin, minlength=nwin_total)
    cnt = cnt_flat.reshape(ncores, windows)
    Gw = np.maximum(1, -(-cnt // 128)).max(axis=0)          # [windows]
    woff = np.concatenate([[0], np.cumsum(Gw)]).astype(np.int64)
    G = int(woff[-1])

    order = np.argsort(gwin, kind="stable")
    gw_sorted = gwin[order]
    grp_start = np.concatenate([[0], np.cumsum(cnt_flat)])
    pos_in_win = np.arange(len(src), dtype=np.int64) - grp_start[gw_sorted]
    tgt = woff[gw_sorted % windows] * 128 + pos_in_win       # per-core slot
    c_sorted = gw_sorted // windows

    # table row of a src node (tables are per-core blocks of `nodes` rows)
    sc = src // own
    src_row = sc * nodes + (src - sc * own)

    # pack (slot, weight) into one uint16: slot<<11 | floor(w*2048).
    # Device dequant: w ~ (wq + 0.5)/2048, abs err <= 1/4096.
    flat = (c_sorted * (G * 128) + tgt).astype(np.int64)
    idx_all = np.zeros(ncores * G * 128, np.int32)
    idx_all[flat] = src_row[order]
    wq = (edge_weight * np.float32(2048.0)).astype(np.int32)
    sw = (slot << 11) | wq
    swp_all = np.zeros(ncores * G * 128, np.uint16)
    swp_all[flat] = sw[order]

    gidx = np.ascontiguousarray(
        idx_all.reshape(ncores, G, 128).transpose(0, 2, 1))
    swp = np.ascontiguousarray(
        swp_all.reshape(ncores, G, 128).transpose(0, 2, 1))

    # ---- x: natural layout, bf16 (truncation); the 25-feature tail is sent
    # pre-transposed so device feat-chunks stay multiples of 128
    xbits = _bf16_trunc_bits(x).reshape(ncores, own, cfg.feat)
    xn = np.zeros((ncores, cfg.nsend, cfg.fpad), np.uint16)
    xn[:, :own, :] = xbits[:, :, : cfg.fpad]
    xn = xn.view(bf16)
    xtailT = np.zeros((ncores, cfg.ftail, cfg.nsend), np.uint16)
    xtailT[:, :, :own] = xbits[:, :, cfg.fpad:].transpose(0, 2, 1)
    xtailT = xtailT.view(bf16)

    # ---- keep mask (0 or 2) as uint8, [tiles, 32 slot, 4 win, 40] layout
    kp = np.zeros((ncores, nodes, hid), np.uint8)
    kb = (dropout_mask_u > 0.5).view(np.uint8)
    kp[:, :own] = (kb + kb).reshape(ncores, own, hid)
    keep4 = np.ascontiguousarray(
        kp.reshape(ncores, tiles, wpt, win, hid).transpose(0, 1, 3, 2, 4)
    ).reshape(ncores, tiles, win, wpt * hid)

    # ---- weights / consts (small; any cast path is fine)
    w1p = np.ascontiguousarray(
        W1[: cfg.fpad].reshape(cfg.kt, 128, hid).transpose(1, 0, 2)
    ).astype(bf16)
    w1t = np.ascontiguousarray(W1[cfg.fpad:]).astype(bf16)
    w2 = W2.astype(np.float32)
    b1r = np.broadcast_to(
        b1.astype(np.float32), (win, 1, hid)).copy()
    b2r = np.broadcast_to(
        b2.astype(np.float32), (128, 1, cfg.ncls)).copy()
    vslot = np.broadcast_to(
        np.arange(win, dtype=np.float32).astype(bf16), (128, 1, win)).copy()

    in_maps = [
        {
            "xn": xn[k],
            "xtailT": xtailT[k],
            "w1p": w1p,
            "w1t": w1t,
            "w2": w2,
            "b1r": b1r,
            "b2r": b2r,
            "vslot": vslot,
            "keep4": keep4[k],
            "gidx": gidx[k],
            "swp": swp[k],
        }
        for k in range(ncores)
    ]
    return in_maps, Gw


# --------------------------------------------------------------------------
# Numpy emulation of the device algorithm (for validation)
# --------------------------------------------------------------------------

def emulate(cfg, in_maps, Gw):
    import ml_dtypes
    f32, f16 = np.float32, ml_dtypes.bfloat16
    ncores, nodes, win, wpt = cfg.ncores, cfg.nodes, cfg.win, cfg.wpt
    hid, ncls, tiles = cfg.hid, cfg.ncls, cfg.tiles
    G = int(Gw.sum())
    woff = np.concatenate([[0], np.cumsum(Gw)])

    # phase A: S1 tables (natural row order; rows >= nsend never gathered)
    s1 = np.zeros((ncores, nodes, hid), f16)
    for k in range(ncores):
        xk = in_maps[k]["xn"].astype(f32)     # [nsend, fpad]
        w1p = in_maps[k]["w1p"].astype(f32)   # [128, kt, hid]
        w1 = w1p.transpose(1, 0, 2).reshape(cfg.fpad, hid)
        xt = in_maps[k]["xtailT"].astype(f32)  # [ftail, nsend]
        w1t = in_maps[k]["w1t"].astype(f32)    # [ftail, hid]
        s1[k, : cfg.nsend] = (xk @ w1 + xt.T @ w1t).astype(f16)
    s1_full = s1.reshape(ncores * nodes, hid)

    def build_onehot(k):
        swp = in_maps[k]["swp"].astype(np.int64)  # [128, G]
        slotb = (swp >> 11).astype(f32)
        wgt = (((swp & 0x7FF).astype(f32) + 0.5) / 2048.0).astype(f16)
        wgt = wgt.astype(f32)
        oh = (slotb[:, :, None] == np.arange(win)[None, None, :])
        return (oh * wgt[:, :, None]).astype(f16).astype(f32)  # [128, G, win]

    def spmm(table, k, oh):
        gidx = in_maps[k]["gidx"]                 # [128, G]
        msg = table[gidx.T].astype(f32)           # [G, 128, hid]
        ohg = oh.transpose(1, 0, 2)               # [G, 128, win]
        agg = np.zeros((tiles, win, wpt, hid), f32)
        for w in range(cfg.windows):
            t, wl = divmod(w, wpt)
            for g in range(woff[w], woff[w + 1]):
                agg[t, :, wl, :] += ohg[g].T @ msg[g]
        return agg                                # [tiles, 32s, 4w, hid]

    h = np.zeros((ncores, nodes, hid), f16)
    for k in range(ncores):
        oh = build_onehot(k)
        agg1 = spmm(s1_full, k, oh)
        b1 = in_maps[k]["b1r"][0, 0]
        keep = in_maps[k]["keep4"].reshape(tiles, win, wpt, hid)
        hb = np.maximum(agg1 + b1, 0.0).astype(f16).astype(f32) * keep
        # natural row order: node (t, w, s) lives at hb[t, s, w]
        h[k] = hb.transpose(0, 2, 1, 3).reshape(nodes, hid).astype(f16)
        in_maps[k]["_oh"] = oh
    h_full = h.reshape(ncores * nodes, hid)

    outs = []
    for k in range(ncores):
        agg2 = spmm(h_full, k, in_maps[k]["_oh"])   # [tiles, 32s, 4w, hid]
        # natural node order: node (t, w, s) -> agg2[t, s, w]
        aggn = agg2.transpose(0, 2, 1, 3).reshape(nodes, hid)
        z = aggn @ in_maps[k]["w2"] + in_maps[k]["b2r"][0, 0]
        m = z.max(1, keepdims=True)
        out = (z - m) - np.log(np.exp(z - m).sum(1, keepdims=True))
        outs.append(out[: cfg.own])
        del in_maps[k]["_oh"]
    return np.concatenate(outs).astype(np.float32)


# --------------------------------------------------------------------------
# Bass/Tile program
# --------------------------------------------------------------------------

def build_program(cfg, Gw, num_devices):
    import concourse.bass as bass
    import concourse.bacc as bacc
    import concourse.mybir as mybir
    import concourse.tile as tile

    f32 = mybir.dt.float32
    bf = mybir.dt.bfloat16
    i32 = mybir.dt.int32
    u16 = mybir.dt.uint16
    u8 = mybir.dt.uint8
    AF = mybir.ActivationFunctionType
    OP = mybir.AluOpType
    X = mybir.AxisListType.X

    G = int(Gw.sum())
    woff = np.concatenate([[0], np.cumsum(Gw)])
    nodes, tiles, win, wpt = cfg.nodes, cfg.tiles, cfg.win, cfg.wpt
    hid, ncls, kt, grp = cfg.hid, cfg.ncls, cfg.kt, cfg.grp
    trows = num_devices * nodes

    nc = bacc.Bacc(
        "TRN2", target_bir_lowering=False, debug=False,
        num_devices=num_devices,
    )

    xn = nc.dram_tensor("xn", [cfg.nsend, cfg.fpad], bf, kind="ExternalInput")
    xtailT = nc.dram_tensor(
        "xtailT", [cfg.ftail, cfg.nsend], bf, kind="ExternalInput")
    w1p = nc.dram_tensor("w1p", [128, kt, hid], bf, kind="ExternalInput")
    w1t = nc.dram_tensor("w1t", [cfg.ftail, hid], bf, kind="ExternalInput")
    w2 = nc.dram_tensor("w2", [hid, ncls], f32, kind="ExternalInput")
    b1r = nc.dram_tensor("b1r", [win, 1, hid], f32, kind="ExternalInput")
    b2r = nc.dram_tensor("b2r", [128, 1, ncls], f32, kind="ExternalInput")
    vslot = nc.dram_tensor("vslot", [128, 1, win], bf, kind="ExternalInput")
    keep4 = nc.dram_tensor(
        "keep4", [tiles, win, wpt * hid], u8, kind="ExternalInput")
    gidx = nc.dram_tensor("gidx", [128, G], i32, kind="ExternalInput")
    swp = nc.dram_tensor("swp", [128, G], u16, kind="ExternalInput")
    out_d = nc.dram_tensor("out", [nodes, ncls], f32, kind="ExternalOutput")

    s1_own = nc.dram_tensor("s1_own", [nodes, hid], bf)
    s1_full = nc.dram_tensor("s1_full", [trows, hid], bf, addr_space="Shared")
    h_own = nc.dram_tensor("h_own", [nodes, hid], bf)
    h_full = nc.dram_tensor("h_full", [trows, hid], bf, addr_space="Shared")

    groups = list(range(num_devices))

    # per-tile group schedule: (g_global, window_in_tile, start, stop)
    sched = []
    for t in range(tiles):
        entries = []
        for wl in range(wpt):
            w = t * wpt + wl
            for g in range(woff[w], woff[w + 1]):
                entries.append(
                    (int(g), wl, g == woff[w], g == woff[w + 1] - 1)
                )
        sched.append(entries)
    rmax = int(max(woff[(t + 1) * wpt] - woff[t * wpt] for t in range(tiles)))

    # phase-A row groups over the nsend sent rows: full groups of `grp`,
    # then one remainder group (row count stays a multiple of 16 for the
    # DMA transpose; the last matmul subtile may be narrower than 128)
    a_groups = []
    n0 = 0
    while n0 < cfg.nsend:
        a_groups.append((n0, min(grp, cfg.nsend - n0)))
        n0 += grp

    with tile.TileContext(nc) as tc:
        with (
            tc.tile_pool(name="const", bufs=1) as constp,
            tc.tile_pool(name="xbuf", bufs=3) as xpool,
            tc.tile_pool(name="psA", bufs=2, space="PSUM") as psA,
            tc.tile_pool(name="s1pc", bufs=3) as spool,
            tc.tile_pool(name="msg", bufs=3) as msgp,
            tc.tile_pool(name="oh", bufs=3) as ohp,
            tc.tile_pool(name="psB", bufs=2, space="PSUM") as psB,
            tc.tile_pool(name="hb", bufs=3) as hpool,
            tc.tile_pool(name="psC", bufs=2, space="PSUM") as psC,
            tc.tile_pool(name="ps2", bufs=2, space="PSUM") as ps2,
            tc.tile_pool(name="ob", bufs=3) as opool,
        ):
            # ---- constants + resident metadata ----
            w1sb = constp.tile([128, kt, hid], bf)
            nc.sync.dma_start(out=w1sb[:], in_=w1p[:])
            w1tsb = constp.tile([cfg.ftail, hid], bf)
            nc.sync.dma_start(out=w1tsb[:], in_=w1t[:])
            xtl = constp.tile([cfg.ftail, cfg.nsend], bf)
            nc.sync.dma_start(out=xtl[:], in_=xtailT[:])
            w2sb = constp.tile([hid, ncls], f32)
            nc.sync.dma_start(out=w2sb[:], in_=w2[:])
            b1sb = constp.tile([win, 1, hid], f32)
            nc.sync.dma_start(out=b1sb[:], in_=b1r[:])
            b2sb = constp.tile([128, 1, ncls], f32)
            nc.sync.dma_start(out=b2sb[:], in_=b2r[:])
            vs = constp.tile([128, 1, win], bf)
            nc.sync.dma_start(out=vs[:], in_=vslot[:])
            gix = constp.tile([128, G], i32)
            nc.sync.dma_start(out=gix[:], in_=gidx[:])
            # unpack swp = slot<<11 | floor(w*2048) into resident bf16 arrays
            swp_sb = constp.tile([128, G], u16)
            nc.sync.dma_start(out=swp_sb[:], in_=swp[:])
            tmp16 = constp.tile([128, G], u16)
            slb = constp.tile([128, G], bf)
            wgb = constp.tile([128, G], bf)
            nc.vector.tensor_scalar(
                out=tmp16[:], in0=swp_sb[:], scalar1=11, scalar2=None,
                op0=OP.logical_shift_right)
            nc.vector.tensor_copy(slb[:], tmp16[:])
            nc.vector.tensor_scalar(
                out=tmp16[:], in0=swp_sb[:], scalar1=0x7FF, scalar2=None,
                op0=OP.bitwise_and)
            nc.scalar.activation(
                out=wgb[:], in_=tmp16[:], func=AF.Copy,
                scale=1.0 / 2048.0, bias=0.5 / 2048.0)
            # keep mask: u8 -> resident bf16 [32, tiles, 4, 40]
            kp8 = constp.tile([win, tiles, wpt * hid], u8)
            nc.sync.dma_start(
                out=kp8[:], in_=keep4[:].rearrange("t p c -> p t c"))
            kpb = constp.tile([win, tiles, wpt, hid], bf)
            nc.vector.tensor_copy(
                kpb[:], kp8[:].rearrange("p t (w c) -> p t w c", w=wpt))

            # ---- phase A: S1_own = (x @ W1) ----
            for (g_n0, g_n) in a_groups:
                xt = xpool.tile([128, kt, grp], bf, tag="xt")
                for k in range(kt):
                    nc.sync.dma_start(
                        out=xt[:, k, :g_n],
                        in_=xn[g_n0:g_n0 + g_n, k * 128:(k + 1) * 128],
                        transpose=True,
                    )
                for sub in range(-(-g_n // 128)):
                    t_n0 = g_n0 + sub * 128
                    t_n = min(128, g_n - sub * 128)
                    ps = psA.tile([128, hid], f32)
                    for k in range(kt):
                        nc.tensor.matmul(
                            ps[:t_n, :],
                            lhsT=xt[:, k, sub * 128:sub * 128 + t_n],
                            rhs=w1sb[:, k, :],
                            start=(k == 0), stop=False,
                        )
                    nc.tensor.matmul(
                        ps[:t_n, :],
                        lhsT=xtl[:, t_n0:t_n0 + t_n],
                        rhs=w1tsb[:],
                        start=False, stop=True,
                    )
                    pc = spool.tile([128, hid], bf, tag="s1pc")
                    nc.vector.tensor_copy(pc[:t_n, :], ps[:t_n, :])
                    nc.sync.dma_start(
                        out=s1_own[t_n0:t_n0 + t_n, :], in_=pc[:t_n, :]
                    )

            # ---- all-gather S1 ----
            nc.gpsimd.collective_compute(
                "AllGather", OP.bypass, replica_groups=[groups],
                ins=[s1_own[:]], outs=[s1_full[:]],
            )

            def gather_and_onehot(t, table, mtag, otag):
                r0 = int(woff[t * wpt])
                rt = int(woff[(t + 1) * wpt]) - r0
                msg = msgp.tile([128, rmax, hid], bf, tag=mtag)
                # funnel the gather's dependencies (WAR on msg) through
                # cheap Pool-engine ops first
                scr = spool.tile([1, 1], i32, tag="scr")
                nc.gpsimd.tensor_copy(scr[:], gix[:1, :1])
                nc.gpsimd.memset(msg[:1, :1, :1], 0.0)
                # HW only supports one offset per partition per indirect DMA
                for r in range(rt):
                    nc.gpsimd.indirect_dma_start(
                        out=msg[:, r, :], out_offset=None,
                        in_=table[:],
                        in_offset=bass.IndirectOffsetOnAxis(
                            ap=gix[:, r0 + r:r0 + r + 1], axis=0
                        ),
                    )
                oht = ohp.tile([128, rmax, win], bf, tag=otag)
                nc.vector.tensor_tensor(
                    out=oht[:, :rt, :],
                    in0=slb[:, r0:r0 + rt].to_broadcast([128, rt, win]),
                    in1=vs[:].to_broadcast([128, rt, win]),
                    op=OP.is_equal,
                )
                nc.vector.tensor_tensor(
                    out=oht[:, :rt, :],
                    in0=oht[:, :rt, :],
                    in1=wgb[:, r0:r0 + rt].to_broadcast([128, rt, win]),
                    op=OP.mult,
                )
                return r0, msg, oht

            # ---- layer 1 SpMM -> h (node-major psum) ----
            for t in range(tiles):
                r0, msg, oht = gather_and_onehot(t, s1_full, "msg1", "oh1")
                ps = psB.tile([win, wpt, hid], f32, tag="agg")
                for (g, wl, st, sp) in sched[t]:
                    r = g - r0
                    nc.tensor.matmul(
                        ps[:, wl, :],
                        lhsT=oht[:, r, :], rhs=msg[:, r, :],
                        start=st, stop=sp,
                    )
                hb = hpool.tile([win, wpt, hid], f32, tag="hb")
                nc.vector.tensor_tensor(
                    out=hb[:], in0=ps[:],
                    in1=b1sb[:].to_broadcast([win, wpt, hid]), op=OP.add,
                )
                nc.scalar.activation(out=hb[:], in_=hb[:], func=AF.Relu)
                hf = hpool.tile([win, wpt, hid], bf, tag="hf")
                nc.vector.tensor_tensor(
                    out=hf[:], in0=hb[:], in1=kpb[:, t], op=OP.mult,
                )
                nc.sync.dma_start(
                    out=h_own[t * 128:(t + 1) * 128, :]
                    .rearrange("(w s) c -> s w c", w=wpt, s=win),
                    in_=hf[:],
                )

            # ---- all-gather h ----
            nc.gpsimd.collective_compute(
                "AllGather", OP.bypass, replica_groups=[groups],
                ins=[h_own[:]], outs=[h_full[:]],
            )

            # ---- layer 2 SpMM (hid-major psum) + @W2 + log_softmax ----
            for t in range(tiles):
                r0, msg, oht = gather_and_onehot(t, h_full, "msg2", "oh2")
                pst = psC.tile([hid, wpt, win], f32, tag="aggT")
                for (g, wl, st, sp) in sched[t]:
                    r = g - r0
                    nc.tensor.matmul(
                        pst[:, wl, :],
                        lhsT=msg[:, r, :], rhs=oht[:, r, :],
                        start=st, stop=sp,
                    )
                at = hpool.tile([hid, wpt, win], f32, tag="at")
                nc.vector.tensor_copy(at[:], pst[:])
                p2 = ps2.tile([128, 1, ncls], f32, tag="s2")
                nc.tensor.matmul(
                    p2[:, 0, :], lhsT=at[:].rearrange("p w s -> p (w s)"),
                    rhs=w2sb[:], start=True, stop=True,
                )
                z = opool.tile([128, 1, ncls], f32, tag="z")
                nc.vector.tensor_tensor(
                    out=z[:], in0=p2[:], in1=b2sb[:], op=OP.add,
                )
                m = opool.tile([128, 1], f32, tag="m")
                nc.vector.tensor_reduce(out=m[:], in_=z[:], axis=X, op=OP.max)
                zc = opool.tile([128, 1, ncls], f32, tag="zc")
                nc.vector.tensor_tensor(
                    out=zc[:], in0=z[:],
                    in1=m[:].to_broadcast([128, 1, ncls]), op=OP.subtract,
                )
                ez = opool.tile([128, 1, ncls], f32, tag="ez")
                nc.scalar.activation(out=ez[:], in_=zc[:], func=AF.Exp)
                s = opool.tile([128, 1], f32, tag="s")
                nc.vector.tensor_reduce(out=s[:], in_=ez[:], axis=X, op=OP.add)
                ls = opool.tile([128, 1], f32, tag="ls")
                nc.scalar.activation(out=ls[:], in_=s[:], func=AF.Ln)
                res = opool.tile([128, 1, ncls], f32, tag="res")
                nc.vector.tensor_tensor(
                    out=res[:], in0=zc[:],
                    in1=ls[:].to_broadcast([128, 1, ncls]), op=OP.subtract,
                )
                nc.sync.dma_start(
                    out=out_d[t * 128:(t + 1) * 128, :], in_=res[:, 0, :]
                )

    nc.compile()
    return nc


# --------------------------------------------------------------------------
# Entry point
# --------------------------------------------------------------------------

def kernel(x, src, dst, edge_weight, W1, b1, W2, b2, dropout_mask_u):
    cfg = CFG
    in_maps, Gw = host_prep(
        cfg, x, src, dst, edge_weight, W1, b1, W2, b2, dropout_mask_u
    )
    nc = build_program(cfg, Gw, cfg.ncores)

    from concourse.bass_utils import run_bass_kernel_spmd

    trace = bool(int(os.environ.get("GNN_TRACE", "0")))
    try:
        res = run_bass_kernel_spmd(
            nc, in_maps, core_ids=list(range(cfg.ncores)), trace=trace
        )
    except ModuleNotFoundError:
        res = run_bass_kernel_spmd(
            nc, in_maps, core_ids=list(range(cfg.ncores)), trace=False
        )
    kernel.last_exec_time_ns = getattr(res, "exec_time_ns", None)
    kernel.last_profile = res
    kernel.last_nc = nc
    kernel.last_in_maps = in_maps
    out = np.concatenate(
        [res.results[k]["out"][: cfg.own] for k in range(cfg.ncores)]
    )
    return out.astype(np.float32)
